# revision 2
# baseline (speedup 1.0000x reference)
"""Discounted cumsum (y[b,h,t,d] = x[b,h,t,d] + gamma[h] * y[b,h,t-1,d]) on 8 trn2 cores.

Blocked parallel scan, pure data parallelism over the B*H=64 (b,h) pairs (8 per core).
SBUF layout per pair: [128 part = t-within-block, 32 blocks x 128 d]. The within-block
scan is one bf16 PE matmul per 4-block group (512 moving columns) against the
triangular gamma-power matrix A[s,t] = gamma^(t-s); accumulation is fp32 in PSUM, so
the end-to-end error is ~2^-9 relative — far inside the 2e-2 gate, which lets the
whole data path run at 2 bytes/element:

  - input x is cast to bf16 on host (halves input traffic vs a hi/lo split)
  - output y is written back as bf16 in the scan layout [t-within-block, block, d]
    (fully contiguous 1 MiB store DMAs); the host un-permutes and upcasts to f32

Cross-block carries: per-block suffix sums r_k = sum_t gamma^(127-t) x[t,k] come from
one u^T X matmul per group; a DMA scatter puts them on 32 partitions; one small f32
matmul with the geometric-decay matrix GT yields the carry-in c_k for every block.
Instead of a rank-1 injection matmul (which would stream all 4096 columns through PE
again), the carry is folded into row 0 of X: x'[0,k] = x[0,k] + gamma*state_{k-1},
via scatter row0 -> DVE add with c -> gather back into row 0. The scan matmul over
the patched X then produces the final y directly.

Host precomputes all gamma-power constants in float64 and pre-transposes the input so
every big DMA is contiguous 8KB lines. Per-pair constants are stacked so the single
SPMD program is gamma-independent (no per-core program specialization).

Walrus allows 1 sync wait on engine instructions / 2 on DMAs; after Tile scheduling,
bass_rust.generate_event_semaphores legalizes by moving excess waits onto
InstEventSemaphore carriers. The tiny bf16 ldweights "absorbers" advance PE's
observed DMA-lane clocks early so hot-path matmuls need at most their one wait.
"""

import numpy as np

B, H, S, D = 4, 16, 4096, 128
T = 128          # block length (matmul contraction dim)
KB = S // T      # 32 blocks per sequence
NG = 4           # blocks per matmul group (4*128 = 512 moving columns)
G = KB // NG     # 8 groups per pair
NCORES = 8
PAIRS = (B * H) // NCORES  # 8 pair-slots per core

_nc_cache = {}


def _build_program():
    if "nc" in _nc_cache:
        return _nc_cache["nc"]

    import concourse.bass as bass
    import concourse.mybir as mybir
    from concourse.tile import TileContext

    f32 = mybir.dt.float32
    bf16 = mybir.dt.bfloat16

    nc = bass.Bass(trn_type="TRN2")

    x_d = nc.declare_dram_parameter("x_all", [PAIRS, T, KB * D], bf16, isOutput=False)
    A_d = nc.declare_dram_parameter("A_all", [T, PAIRS * T], bf16, isOutput=False)
    u_d = nc.declare_dram_parameter("u_all", [T, PAIRS], bf16, isOutput=False)
    GT_d = nc.declare_dram_parameter("GT_all", [KB, PAIRS * KB], f32, isOutput=False)
    y_d = nc.declare_dram_parameter("y", [PAIRS, T, KB * D], bf16, isOutput=True)

    with TileContext(nc) as tc:
        with (
            tc.tile_pool(name="const", bufs=1) as cpool,
            tc.tile_pool(name="xin", bufs=3) as xpool,
            tc.tile_pool(name="yout", bufs=2) as ypool,
            tc.tile_pool(name="rfl", bufs=2) as rfpool,
            tc.tile_pool(name="r32", bufs=4) as r32pool,
            tc.tile_pool(name="x0s", bufs=4) as x0pool,
            tc.tile_pool(name="grp_ps", bufs=4, space="PSUM") as gp_pool,
            tc.tile_pool(name="mmr_ps", bufs=2, space="PSUM") as rp_pool,
            tc.tile_pool(name="c_ps", bufs=2, space="PSUM") as cp_pool,
        ):
            Ac = cpool.tile([T, PAIRS * T], bf16, tag="Ac")
            uc = cpool.tile([T, PAIRS], bf16, tag="uc")
            GTc = cpool.tile([KB, PAIRS * KB], f32, tag="GTc")
            nc.gpsimd.dma_start(out=Ac[:], in_=A_d[:])
            nc.gpsimd.dma_start(out=uc[:], in_=u_d[:])
            nc.gpsimd.dma_start(out=GTc[:], in_=GT_d[:])

            def absorb(ap_src):
                # standalone bf16 ldweights: makes PE wait on that tile's DMA
                # lane here; the real matmuls self-load their own stationary.
                nc.tensor.ldweights(ap_src.bitcast(bf16))

            absorb(Ac[0:1, 0:1])
            absorb(uc[0:1, 0:1])
            absorb(GTc[0:1, 0:1].bitcast(bf16))

            for p in range(PAIRS):
                # ---- load pair (pre-transposed on host: contiguous rows)
                X = xpool.tile([T, KB * D], bf16, tag="X")
                nc.sync.dma_start(out=X[:], in_=x_d[p])
                absorb(X[0:1, 0:1])

                # ---- block suffix sums r_k = sum_t gamma^(127-t) x[t,k]
                Rflat = rfpool.tile([1, KB * D], f32, tag="Rflat")
                for g in range(G):
                    sl = slice(g * NG * D, (g + 1) * NG * D)
                    rp = rp_pool.tile([1, NG * D], f32, tag="rp")
                    nc.tensor.matmul(
                        rp[:], lhsT=uc[:, p : p + 1], rhs=X[:, sl],
                        start=True, stop=True,
                    )
                    if g < 4:
                        nc.vector.tensor_copy(out=Rflat[:, sl], in_=rp[:])
                    else:
                        nc.scalar.copy(out=Rflat[:, sl], in_=rp[:])
                # scatter [1,(k d)] -> [KB part, d] on the SP ring (the ACT
                # ring carries the big out-DMAs whose descriptor generation
                # would delay this chain-critical transfer). Flat orders zip.
                R32 = r32pool.tile([KB, D], f32, tag="R32")
                nc.sync.dma_start(out=R32[:], in_=Rflat[:])

                # ---- carries: cp[k] = gamma * state at end of block k-1
                # (the gamma factor is folded into GT host-side)
                cp = cp_pool.tile([KB, D], f32, tag="cp")
                nc.tensor.matmul(
                    cp[:], lhsT=GTc[:, p * KB : (p + 1) * KB], rhs=R32[:],
                    start=True, stop=True,
                )
                # ---- patch row 0 of X: x'[0,k] = x[0,k] + cp[k]
                X0s = x0pool.tile([KB, D], bf16, tag="X0s")
                nc.sync.dma_start(out=X0s[:], in_=X[0:1, :])
                X0n = x0pool.tile([KB, D], bf16, tag="X0n")
                nc.vector.tensor_tensor(
                    out=X0n[:], in0=cp[:], in1=X0s[:],
                    op=mybir.AluOpType.add,
                )
                nc.sync.dma_start(out=X[0:1, :], in_=X0n[:])
                absorb(X[0:1, 0:1])

                # ---- per group: within-block scan over the patched X
                Ys = ypool.tile([T, KB * D], bf16, tag="Ys")
                Ap = Ac[:, p * T : (p + 1) * T]
                for g in range(G):
                    grp = gp_pool.tile([T, NG * D], f32, tag="grp")
                    sl = slice(g * NG * D, (g + 1) * NG * D)
                    nc.tensor.matmul(
                        grp[:], lhsT=Ap, rhs=X[:, sl], start=True, stop=True,
                    )
                    if g % 2 == 0:
                        nc.vector.tensor_copy(out=Ys[:, sl], in_=grp[:])
                    else:
                        nc.scalar.copy(out=Ys[:, sl], in_=grp[:])

                # ---- store pair (contiguous; host un-permutes)
                nc.scalar.dma_start(out=y_d[p], in_=Ys[:])

    # Split excess per-instruction sync waits onto InstEventSemaphore carriers.
    import bass_rust

    bass_rust.generate_event_semaphores(nc)

    _nc_cache["nc"] = nc
    return nc


def _host_constants(g):
    """Per-pair constants from float64 gamma powers."""
    pw = np.power(g, np.arange(2 * S, dtype=np.float64))
    t_idx = np.arange(T)
    t_minus_s = t_idx[None, :] - t_idx[:, None]
    A = np.where(t_minus_s >= 0, pw[np.clip(t_minus_s, 0, None)], 0.0)
    u = pw[127 - t_idx]
    # carry into block k from block j's suffix sum: gamma^(128(k-1-j)+1), j<k
    k_minus_j = np.arange(KB)[None, :] - 1 - np.arange(KB)[:, None]
    pw128 = np.power(pw[T], np.arange(KB, dtype=np.float64))
    GT = np.where(k_minus_j >= 0, pw128[np.clip(k_minus_j, 0, None)], 0.0) * g
    return A, u, GT


def _make_in_maps(tensor, gamma):
    import ml_dtypes

    bf16 = ml_dtypes.bfloat16
    x = np.ascontiguousarray(np.asarray(tensor, dtype=np.float32)).reshape(
        B * H, S, D
    )
    gam = np.asarray(gamma, dtype=np.float64).reshape(H)

    in_maps = []
    for c in range(NCORES):
        xa = np.empty((PAIRS, T, KB * D), bf16)
        A_all = np.empty((T, PAIRS * T), bf16)
        u_all = np.empty((T, PAIRS), bf16)
        GT_all = np.empty((KB, PAIRS * KB), np.float32)
        for p in range(PAIRS):
            pid = c * PAIRS + p
            g = gam[pid % H]
            A, u, GT = _host_constants(g)
            # x in scan layout [s, (k, d)]
            xa[p] = (
                x[pid]
                .reshape(KB, T, D)
                .transpose(1, 0, 2)
                .reshape(T, KB * D)
                .astype(bf16)
            )
            A_all[:, p * T : (p + 1) * T] = A.astype(bf16)
            u_all[:, p] = u.astype(bf16)
            GT_all[:, p * KB : (p + 1) * KB] = GT.astype(np.float32)
        in_maps.append(
            {"x_all": xa, "A_all": A_all, "u_all": u_all, "GT_all": GT_all}
        )
    return in_maps


def kernel(tensor, gamma):
    from concourse.bass_utils import run_bass_kernel_spmd

    in_maps = _make_in_maps(tensor, gamma)
    nc = _build_program()
    res = run_bass_kernel_spmd(nc, in_maps, list(range(NCORES))).results
    y = np.empty((B * H, S, D), np.float32)
    for c in range(NCORES):
        yc = np.asarray(res[c]["y"]).astype(np.float32)
        y[c * PAIRS : (c + 1) * PAIRS] = (
            yc.reshape(PAIRS, T, KB, D).transpose(0, 2, 1, 3).reshape(PAIRS, S, D)
        )
    return y.reshape(B, H, S, D)


# revision 4
# speedup vs baseline: 1.4777x; 1.4777x over previous
"""Discounted cumsum (y[b,h,t,d] = x[b,h,t,d] + gamma[h] * y[b,h,t-1,d]) on 8 trn2 cores.

Blocked parallel scan, pure data parallelism over the B*H=64 (b,h) pairs (8 per core).
SBUF layout per pair: [128 part = t-within-block, 32 blocks x 128 d]. The within-block
scan is one bf16 PE matmul per 4-block group (512 moving columns) against the
triangular gamma-power matrix A[s,t] = gamma^(t-s); accumulation is fp32 in PSUM, so
the end-to-end error is ~2^-9 relative - far inside the 2e-2 gate, which lets the
whole data path run at 2 bytes/element: input x is cast to bf16 on host, output y is
written back as bf16 in the scan layout [t-within-block, block, d] (fully contiguous
1 MiB DMAs both ways); the host un-permutes and upcasts.

Cross-block carries with almost no PSUM->SBUF copy traffic: the per-block suffix
sums r_k = sum_t gamma^(127-t) x[t,k] are computed TRANSPOSED - 32 one-column
matmuls (block slab of X as the stationary, u as the moving vector) fill Rt[d, k],
so only [128,32]+[32,128] copies and one PE transpose stand between PSUM and the
carry matmul c = GT^T r (all f32). The carry-in c_k is folded into row 0 of X
(x'[0,k] = x[0,k] + gamma*state_{k-1}) via a pre-staged row-0 scatter, a DVE add,
and a gather back into X - so the scan matmul over the patched X produces final y
directly, with no rank-1 injection pass and no flat-row blocksum assembly.

The per-pair chain (blocksums -> copy -> transpose -> copy -> carry mm -> add ->
gather) has a few us of latency, so the PE stream is software-pipelined: the carry
matmul of pair p issues one pair behind its transpose, the scan matmuls two pairs
behind, and all 8 input loads issue up front (xin bufs=8) so the input queue
streams back-to-back while PE/DVE/ACT stay busy.

Walrus allows 1 sync wait on engine instructions / 2 on DMAs; after Tile scheduling,
bass_rust.generate_event_semaphores legalizes by moving excess waits onto
InstEventSemaphore carriers. The tiny bf16 ldweights "absorbers" advance PE's
observed DMA-lane clocks early so hot-path matmuls need at most their one wait.
"""

import numpy as np

B, H, S, D = 4, 16, 4096, 128
T = 128          # block length (matmul contraction dim)
KB = S // T      # 32 blocks per sequence
NG = 4           # blocks per scan-matmul group (4*128 = 512 moving columns)
G = KB // NG     # 8 scan groups per pair
DEPTH = 2        # software-pipeline depth (scan of pair p after chain of p+DEPTH)
NCORES = 8
PAIRS = (B * H) // NCORES  # 8 pair-slots per core

_nc_cache = {}


def _build_program():
    if "nc" in _nc_cache:
        return _nc_cache["nc"]

    import concourse.bass as bass
    import concourse.mybir as mybir
    from concourse.tile import TileContext

    f32 = mybir.dt.float32
    bf16 = mybir.dt.bfloat16

    nc = bass.Bass(trn_type="TRN2")

    x_d = nc.declare_dram_parameter("x_all", [PAIRS, T, KB * D], bf16, isOutput=False)
    A_d = nc.declare_dram_parameter("A_all", [T, PAIRS * T], bf16, isOutput=False)
    u_d = nc.declare_dram_parameter("u_all", [T, PAIRS], bf16, isOutput=False)
    GT_d = nc.declare_dram_parameter("GT_all", [KB, PAIRS * KB], f32, isOutput=False)
    I_d = nc.declare_dram_parameter("I_all", [T, T], f32, isOutput=False)
    y_d = nc.declare_dram_parameter("y", [PAIRS, T, KB * D], bf16, isOutput=True)

    with TileContext(nc) as tc:
        with (
            tc.tile_pool(name="const", bufs=1) as cpool,
            tc.tile_pool(name="xin", bufs=PAIRS) as xpool,
            tc.tile_pool(name="yout", bufs=3) as ypool,
            tc.tile_pool(name="rtsb", bufs=2) as rtsbpool,
            tc.tile_pool(name="r32sb", bufs=2) as r32sbpool,
            tc.tile_pool(name="x0s", bufs=PAIRS) as x0spool,
            tc.tile_pool(name="x0n", bufs=2) as x0npool,
            tc.tile_pool(name="grp_ps", bufs=2, space="PSUM") as gp_pool,
            tc.tile_pool(name="rt_ps", bufs=2, space="PSUM") as rt_pool,
            tc.tile_pool(name="r32_ps", bufs=2, space="PSUM") as r32_pool,
            tc.tile_pool(name="c_ps", bufs=2, space="PSUM") as cp_pool,
        ):
            Ac = cpool.tile([T, PAIRS * T], bf16, tag="Ac")
            uc = cpool.tile([T, PAIRS], bf16, tag="uc")
            GTc = cpool.tile([KB, PAIRS * KB], f32, tag="GTc")
            Ic = cpool.tile([T, T], f32, tag="Ic")
            nc.gpsimd.dma_start(out=Ac[:], in_=A_d[:])
            nc.gpsimd.dma_start(out=uc[:], in_=u_d[:])
            nc.gpsimd.dma_start(out=GTc[:], in_=GT_d[:])
            nc.gpsimd.dma_start(out=Ic[:], in_=I_d[:])

            def absorb(ap_src):
                # standalone bf16 ldweights: makes PE wait on that tile's DMA
                # lane here; the real matmuls self-load their own stationary.
                nc.tensor.ldweights(ap_src.bitcast(bf16))

            absorb(Ac[0:1, 0:1])
            absorb(uc[0:1, 0:1])
            absorb(GTc[0:1, 0:1].bitcast(bf16))
            absorb(Ic[0:1, 0:1].bitcast(bf16))

            # ---- all input loads up front: the queue streams back-to-back
            Xs = []
            for p in range(PAIRS):
                X = xpool.tile([T, KB * D], bf16, tag="X")
                nc.sync.dma_start(out=X[:], in_=x_d[p])
                absorb(X[0:1, 0:1])
                Xs.append(X)
            # row 0 of each pair onto 32 partitions (ready for the carry add)
            X0ss = []
            for p in range(PAIRS):
                X0s = x0spool.tile([KB, D], bf16, tag="X0s")
                nc.gpsimd.dma_start(out=X0s[:], in_=Xs[p][0:1, :])
                X0ss.append(X0s)

            R32s = [None] * PAIRS

            def stage1(p):
                """Transposed blocksums: Rt[d,k] via 32 one-column matmuls."""
                X = Xs[p]
                Rt = rt_pool.tile([D, KB], f32, tag="Rt")
                for k in range(KB):
                    nc.tensor.matmul(
                        Rt[:, k : k + 1], lhsT=X[:, k * D : (k + 1) * D],
                        rhs=uc[:, p : p + 1], start=True, stop=True,
                    )
                Rtsb = rtsbpool.tile([D, KB], f32, tag="Rtsb")
                nc.vector.tensor_copy(out=Rtsb[:], in_=Rt[:])
                R32p = r32_pool.tile([KB, D], f32, tag="R32p")
                nc.tensor.transpose(R32p[:], Rtsb[:], Ic[:])
                R32 = r32sbpool.tile([KB, D], f32, tag="R32")
                nc.scalar.copy(out=R32[:], in_=R32p[:])
                R32s[p] = R32

            def stage1b(p):
                """Carry matmul and the row-0 patch of X."""
                # carry-in c_k = gamma * state at end of block k-1
                cp = cp_pool.tile([KB, D], f32, tag="cp")
                nc.tensor.matmul(
                    cp[:], lhsT=GTc[:, p * KB : (p + 1) * KB], rhs=R32s[p][:],
                    start=True, stop=True,
                )
                # patch row 0 of X: x'[0,k] = x[0,k] + c_k
                X0n = x0npool.tile([KB, D], bf16, tag="X0n")
                nc.vector.tensor_tensor(
                    out=X0n[:], in0=cp[:], in1=X0ss[p][:],
                    op=mybir.AluOpType.add,
                )
                nc.gpsimd.dma_start(out=Xs[p][0:1, :], in_=X0n[:])
                absorb(Xs[p][0:1, 0:1])

            def stage2(p):
                """Within-block scan over the patched X, copy-out, store."""
                X = Xs[p]
                Ys = ypool.tile([T, KB * D], bf16, tag="Ys")
                Ap = Ac[:, p * T : (p + 1) * T]
                for g in range(G):
                    grp = gp_pool.tile([T, NG * D], f32, tag="grp")
                    sl = slice(g * NG * D, (g + 1) * NG * D)
                    nc.tensor.matmul(
                        grp[:], lhsT=Ap, rhs=X[:, sl], start=True, stop=True,
                    )
                    if g % 2 == 0:
                        nc.vector.tensor_copy(out=Ys[:, sl], in_=grp[:])
                    else:
                        nc.scalar.copy(out=Ys[:, sl], in_=grp[:])
                nc.scalar.dma_start(out=y_d[p], in_=Ys[:])

            for p in range(PAIRS + DEPTH):
                if p < PAIRS:
                    stage1(p)
                if 1 <= p < PAIRS + 1:
                    stage1b(p - 1)
                if p >= DEPTH:
                    stage2(p - DEPTH)

    # Split excess per-instruction sync waits onto InstEventSemaphore carriers.
    import bass_rust

    bass_rust.generate_event_semaphores(nc)

    _nc_cache["nc"] = nc
    return nc


def _host_constants(g):
    """Per-pair constants from float64 gamma powers."""
    pw = np.power(g, np.arange(2 * S, dtype=np.float64))
    t_idx = np.arange(T)
    t_minus_s = t_idx[None, :] - t_idx[:, None]
    A = np.where(t_minus_s >= 0, pw[np.clip(t_minus_s, 0, None)], 0.0)
    u = pw[127 - t_idx]
    # carry into block k' from block k's suffix sum: gamma^(128(k'-1-k)+1), k<k'
    k_idx = np.arange(KB)
    kp_minus_k = k_idx[None, :] - 1 - k_idx[:, None]
    pw128 = np.power(pw[T], np.arange(KB, dtype=np.float64))
    GT = np.where(kp_minus_k >= 0, pw128[np.clip(kp_minus_k, 0, None)], 0.0) * g
    return A, u, GT


def _make_in_maps(tensor, gamma):
    import ml_dtypes

    bf16 = ml_dtypes.bfloat16
    x = np.ascontiguousarray(np.asarray(tensor, dtype=np.float32)).reshape(
        B * H, S, D
    )
    gam = np.asarray(gamma, dtype=np.float64).reshape(H)

    in_maps = []
    for c in range(NCORES):
        xa = np.empty((PAIRS, T, KB * D), bf16)
        A_all = np.empty((T, PAIRS * T), bf16)
        u_all = np.empty((T, PAIRS), bf16)
        GT_all = np.empty((KB, PAIRS * KB), np.float32)
        for p in range(PAIRS):
            pid = c * PAIRS + p
            g = gam[pid % H]
            A, u, GT = _host_constants(g)
            # x in scan layout [s, (k, d)]
            xa[p] = (
                x[pid]
                .reshape(KB, T, D)
                .transpose(1, 0, 2)
                .reshape(T, KB * D)
                .astype(bf16)
            )
            A_all[:, p * T : (p + 1) * T] = A.astype(bf16)
            u_all[:, p] = u.astype(bf16)
            GT_all[:, p * KB : (p + 1) * KB] = GT.astype(np.float32)
        in_maps.append(
            {
                "x_all": xa,
                "A_all": A_all,
                "u_all": u_all,
                "GT_all": GT_all,
                "I_all": np.eye(T, dtype=np.float32),
            }
        )
    return in_maps


def kernel(tensor, gamma):
    from concourse.bass_utils import run_bass_kernel_spmd

    in_maps = _make_in_maps(tensor, gamma)
    nc = _build_program()
    res = run_bass_kernel_spmd(nc, in_maps, list(range(NCORES))).results
    y = np.empty((B * H, S, D), np.float32)
    for c in range(NCORES):
        yc = np.asarray(res[c]["y"]).astype(np.float32)
        y[c * PAIRS : (c + 1) * PAIRS] = (
            yc.reshape(PAIRS, T, KB, D).transpose(0, 2, 1, 3).reshape(PAIRS, S, D)
        )
    return y.reshape(B, H, S, D)


# revision 6
# speedup vs baseline: 1.5537x; 1.0514x over previous
"""Discounted cumsum (y[b,h,t,d] = x[b,h,t,d] + gamma[h] * y[b,h,t-1,d]) on 8 trn2 cores.

Blocked parallel scan, pure data parallelism over the B*H=64 (b,h) pairs (8 per core).
SBUF layout per pair: [128 part = t-within-block, 32 blocks x 128 d]. The within-block
scan is one bf16 PE matmul per 4-block group (512 moving columns) against the
triangular gamma-power matrix A[s,t] = gamma^(t-s); accumulation is fp32 in PSUM, so
the end-to-end error is ~2^-9 relative - far inside the 2e-2 gate, which lets the
whole data path run at 2 bytes/element: input x is cast to bf16 on host, output y is
written back as bf16 in the scan layout [t-within-block, block, d] (fully contiguous
1 MiB DMAs both ways); the host un-permutes and upcasts.

Carries come straight out of X with no intermediate block-sum tensor: the patched
row 0 (x'[0,k] = x[0,k] + gamma*state_{k-1}) is ONE accumulation group of 32
128-column matmuls, cp[k',d] = sum_k sum_t W_k[t,k'] x[t,(k,d)], where
W_k[t,k'] = gamma^(127-t) * gamma^(128(k'-k-1)+1) for k'>k is Toeplitz in (k'-k):
every W_k is a sliding 32-column window of one tiny per-pair constant
Wbig[t, m] = gamma^(127-t+128(m-32)+1), and the x0 term rides along as a spike
Wbig[0,31] = 1. One bf16 copy of cp and a gather back into X row 0 later, the scan
matmul over the patched X produces final y directly.

Input loads are split in half across the sync and gpsimd DMA queues, stores
alternate between the scalar and sync queues, so input and output each stream on
two queues and overlap. All loads issue up front (xin bufs=8); the PE stream is
software-pipelined (scans of pair p issue after the carry chain of pair p+2) so PE
never waits on the gather round-trip.

Walrus allows 1 sync wait on engine instructions / 2 on DMAs; after Tile scheduling,
bass_rust.generate_event_semaphores legalizes by moving excess waits onto
InstEventSemaphore carriers. The tiny bf16 ldweights "absorbers" advance PE's
observed DMA-lane clocks early so hot-path matmuls need at most their one wait.
"""

import numpy as np

B, H, S, D = 4, 16, 4096, 128
T = 128          # block length (matmul contraction dim)
KB = S // T      # 32 blocks per sequence
NG = 4           # blocks per scan-matmul group (4*128 = 512 moving columns)
G = KB // NG     # 8 scan groups per pair
WW = 2 * KB - 1  # sliding-window width of the carry constant
DEPTH = 2        # software-pipeline depth (scan of pair p after chain of p+DEPTH)
NCORES = 8
PAIRS = (B * H) // NCORES  # 8 pair-slots per core

_nc_cache = {}


def _build_program():
    if "nc" in _nc_cache:
        return _nc_cache["nc"]

    import concourse.bass as bass
    import concourse.mybir as mybir
    from concourse.tile import TileContext

    f32 = mybir.dt.float32
    bf16 = mybir.dt.bfloat16

    nc = bass.Bass(trn_type="TRN2")

    x_d = nc.declare_dram_parameter("x_all", [PAIRS, T, KB * D], bf16, isOutput=False)
    A_d = nc.declare_dram_parameter("A_all", [T, PAIRS * T], bf16, isOutput=False)
    W_d = nc.declare_dram_parameter("W_all", [T, PAIRS * WW], bf16, isOutput=False)
    y_d = nc.declare_dram_parameter("y", [PAIRS, T, KB * D], bf16, isOutput=True)

    with TileContext(nc) as tc:
        with (
            tc.tile_pool(name="const", bufs=1) as cpool,
            tc.tile_pool(name="xin", bufs=PAIRS) as xpool,
            tc.tile_pool(name="yout", bufs=3) as ypool,
            tc.tile_pool(name="x0n", bufs=2) as x0npool,
            tc.tile_pool(name="grp_ps", bufs=6, space="PSUM") as gp_pool,
            tc.tile_pool(name="c_ps", bufs=2, space="PSUM") as cp_pool,
        ):
            Ac = cpool.tile([T, PAIRS * T], bf16, tag="Ac")
            Wc = cpool.tile([T, PAIRS * WW], bf16, tag="Wc")
            nc.sync.dma_start(out=Ac[:], in_=A_d[:])
            nc.sync.dma_start(out=Wc[:], in_=W_d[:])

            def absorb(ap_src):
                # standalone bf16 ldweights: makes PE wait on that tile's DMA
                # lane here; the real matmuls self-load their own stationary.
                nc.tensor.ldweights(ap_src.bitcast(bf16))

            absorb(Ac[0:1, 0:1])
            absorb(Wc[0:1, 0:1])

            # ---- all input loads up front, halves on two queues
            Xs = []
            half = T // 2
            for p in range(PAIRS):
                X = xpool.tile([T, KB * D], bf16, tag="X")
                nc.sync.dma_start(out=X[0:half, :], in_=x_d[p, 0:half])
                nc.gpsimd.dma_start(out=X[half:T, :], in_=x_d[p, half:T])
                absorb(X[0:1, 0:1])
                Xs.append(X)

            def stage1(p):
                """Patched row 0 via the sliding-window carry accumulation."""
                X = Xs[p]
                Wp = Wc[:, p * WW : (p + 1) * WW]
                cp = cp_pool.tile([KB, D], f32, tag="cp")
                for k in range(KB):
                    nc.tensor.matmul(
                        cp[:], lhsT=Wp[:, KB - 1 - k : 2 * KB - 1 - k],
                        rhs=X[:, k * D : (k + 1) * D],
                        start=(k == 0), stop=(k == KB - 1),
                    )
                X0n = x0npool.tile([KB, D], bf16, tag="X0n")
                if p % 2 == 0:
                    nc.vector.tensor_copy(out=X0n[:], in_=cp[:])
                else:
                    nc.scalar.copy(out=X0n[:], in_=cp[:])
                nc.gpsimd.dma_start(out=X[0:1, :], in_=X0n[:])
                absorb(X[0:1, 0:1])

            def stage2(p):
                """Within-block scan over the patched X, copy-out, store."""
                X = Xs[p]
                Ys = ypool.tile([T, KB * D], bf16, tag="Ys")
                Ap = Ac[:, p * T : (p + 1) * T]
                for g in range(G):
                    grp = gp_pool.tile([T, NG * D], f32, tag="grp")
                    sl = slice(g * NG * D, (g + 1) * NG * D)
                    nc.tensor.matmul(
                        grp[:], lhsT=Ap, rhs=X[:, sl], start=True, stop=True,
                    )
                    if g % 2 == 0:
                        nc.vector.tensor_copy(out=Ys[:, sl], in_=grp[:])
                    else:
                        nc.scalar.copy(out=Ys[:, sl], in_=grp[:])
                if p % 2 == 0:
                    nc.scalar.dma_start(out=y_d[p], in_=Ys[:])
                else:
                    nc.sync.dma_start(out=y_d[p], in_=Ys[:])

            for p in range(PAIRS + DEPTH):
                if p < PAIRS:
                    stage1(p)
                if p >= DEPTH:
                    stage2(p - DEPTH)

    # Split excess per-instruction sync waits onto InstEventSemaphore carriers.
    import bass_rust

    bass_rust.generate_event_semaphores(nc)

    _nc_cache["nc"] = nc
    return nc


def _host_constants(g):
    """Per-pair constants from float64 gamma powers."""
    pw = np.power(g, np.arange(2 * S, dtype=np.float64))
    t_idx = np.arange(T)
    t_minus_s = t_idx[None, :] - t_idx[:, None]
    A = np.where(t_minus_s >= 0, pw[np.clip(t_minus_s, 0, None)], 0.0)
    # sliding-window carry constant: for the block-k matmul, columns
    # [KB-1-k, 2KB-1-k) give W_k[t,k'] = gamma^(127-t) * gamma^(128(k'-k-1)+1)
    # for k'>k (m = k'-k+KB-1 >= KB), plus the x0 spike at (t=0, m=KB-1).
    m_idx = np.arange(WW)
    Wbig = np.zeros((T, WW))
    vals = pw[127 - t_idx][:, None] * np.where(
        m_idx[None, :] >= KB, pw[np.clip(T * (m_idx[None, :] - KB) + 1, 0, None)], 0.0
    )
    Wbig[:, :] = vals
    Wbig[0, KB - 1] = 1.0
    return A, Wbig


def _make_in_maps(tensor, gamma):
    import ml_dtypes

    bf16 = ml_dtypes.bfloat16
    x = np.ascontiguousarray(np.asarray(tensor, dtype=np.float32)).reshape(
        B * H, S, D
    )
    gam = np.asarray(gamma, dtype=np.float64).reshape(H)

    in_maps = []
    for c in range(NCORES):
        xa = np.empty((PAIRS, T, KB * D), bf16)
        A_all = np.empty((T, PAIRS * T), bf16)
        W_all = np.empty((T, PAIRS * WW), bf16)
        for p in range(PAIRS):
            pid = c * PAIRS + p
            g = gam[pid % H]
            A, Wbig = _host_constants(g)
            # x in scan layout [s, (k, d)]
            xa[p] = (
                x[pid]
                .reshape(KB, T, D)
                .transpose(1, 0, 2)
                .reshape(T, KB * D)
                .astype(bf16)
            )
            A_all[:, p * T : (p + 1) * T] = A.astype(bf16)
            W_all[:, p * WW : (p + 1) * WW] = Wbig.astype(bf16)
        in_maps.append({"x_all": xa, "A_all": A_all, "W_all": W_all})
    return in_maps


def kernel(tensor, gamma):
    from concourse.bass_utils import run_bass_kernel_spmd

    in_maps = _make_in_maps(tensor, gamma)
    nc = _build_program()
    res = run_bass_kernel_spmd(nc, in_maps, list(range(NCORES))).results
    y = np.empty((B * H, S, D), np.float32)
    for c in range(NCORES):
        yc = np.asarray(res[c]["y"]).astype(np.float32)
        y[c * PAIRS : (c + 1) * PAIRS] = (
            yc.reshape(PAIRS, T, KB, D).transpose(0, 2, 1, 3).reshape(PAIRS, S, D)
        )
    return y.reshape(B, H, S, D)


# revision 10
# speedup vs baseline: 1.5587x; 1.0032x over previous
"""Discounted cumsum (y[b,h,t,d] = x[b,h,t,d] + gamma[h] * y[b,h,t-1,d]) on 8 trn2 cores.

Blocked parallel scan, pure data parallelism over the B*H=64 (b,h) pairs (8 per core).
SBUF layout per pair: [128 part = t-within-block, 32 blocks x 128 d]. The within-block
scan is one bf16 PE matmul per 4-block group (512 moving columns) against the
triangular gamma-power matrix A[s,t] = gamma^(t-s); accumulation is fp32 in PSUM, so
the end-to-end error is ~2^-9 relative - far inside the 2e-2 gate, which lets the
whole data path run at 2 bytes/element: input x is cast to bf16 on host, output y is
written back as bf16 in the scan layout [t-within-block, block, d] (fully contiguous
1 MiB DMAs both ways); the host un-permutes and upcasts.

Carries come straight out of X with no intermediate block-sum tensor: the patched
row 0 (x'[0,k] = x[0,k] + gamma*state_{k-1}) is ONE accumulation group of 32
128-column matmuls, cp[k',d] = sum_k sum_t W_k[t,k'] x[t,(k,d)], where
W_k[t,k'] = gamma^(127-t) * gamma^(128(k'-k-1)+1) for k'>k is Toeplitz in (k'-k):
every W_k is a sliding 32-column window of one tiny per-pair constant
Wbig[t, m] = gamma^(127-t+128(m-32)+1), and the x0 term rides along as a spike
Wbig[0,31] = 1. One bf16 copy of cp and a gather back into X row 0 later, the scan
matmul over the patched X produces final y directly.

Input loads are split in half across the sync and gpsimd DMA queues, stores
alternate between the scalar and sync queues, so input and output each stream on
two queues and overlap. All loads issue up front (xin bufs=8); the PE stream is
software-pipelined (scans of pair p issue after the carry chain of pair p+2) so PE
never waits on the gather round-trip.

Walrus allows 1 sync wait on engine instructions / 2 on DMAs; after Tile scheduling,
bass_rust.generate_event_semaphores legalizes by moving excess waits onto
InstEventSemaphore carriers. The tiny bf16 ldweights "absorbers" advance PE's
observed DMA-lane clocks early so hot-path matmuls need at most their one wait.
"""

import numpy as np

B, H, S, D = 4, 16, 4096, 128
T = 128          # block length (matmul contraction dim)
KB = S // T      # 32 blocks per sequence
NG = 4           # blocks per scan-matmul group (4*128 = 512 moving columns)
G = KB // NG     # 8 scan groups per pair
WW = 2 * KB - 1  # sliding-window width of the carry constant
DEPTH = 2        # software-pipeline depth (scan of pair p after chain of p+DEPTH)
NCORES = 8
PAIRS = (B * H) // NCORES  # 8 pair-slots per core

_nc_cache = {}


def _build_program():
    if "nc" in _nc_cache:
        return _nc_cache["nc"]

    import concourse.bass as bass
    import concourse.mybir as mybir
    from concourse.tile import TileContext

    f32 = mybir.dt.float32
    bf16 = mybir.dt.bfloat16

    nc = bass.Bass(trn_type="TRN2")

    x_d = nc.declare_dram_parameter("x_all", [PAIRS, T, KB * D], bf16, isOutput=False)
    A_d = nc.declare_dram_parameter("A_all", [T, PAIRS * T], bf16, isOutput=False)
    W_d = nc.declare_dram_parameter("W_all", [T, PAIRS * WW], bf16, isOutput=False)
    y_d = nc.declare_dram_parameter("y", [PAIRS, T, KB * D], bf16, isOutput=True)

    with TileContext(nc) as tc:
        with (
            tc.tile_pool(name="const", bufs=1) as cpool,
            tc.tile_pool(name="xin", bufs=PAIRS) as xpool,
            tc.tile_pool(name="yout", bufs=3) as ypool,
            tc.tile_pool(name="x0n", bufs=2) as x0npool,
            tc.tile_pool(name="grp_ps", bufs=6, space="PSUM") as gp_pool,
            tc.tile_pool(name="c_ps", bufs=2, space="PSUM") as cp_pool,
        ):
            Ac = cpool.tile([T, PAIRS * T], bf16, tag="Ac")
            Wc = cpool.tile([T, PAIRS * WW], bf16, tag="Wc")
            nc.sync.dma_start(out=Wc[:], in_=W_d[:])
            nc.scalar.dma_start(out=Ac[:], in_=A_d[:])

            def absorb(ap_src):
                # standalone bf16 ldweights: makes PE wait on that tile's DMA
                # lane here; the real matmuls self-load their own stationary.
                nc.tensor.ldweights(ap_src.bitcast(bf16))

            absorb(Ac[0:1, 0:1])
            absorb(Wc[0:1, 0:1])

            # ---- all input loads up front, halves on two queues
            Xs = []
            half = T // 2
            for p in range(PAIRS):
                X = xpool.tile([T, KB * D], bf16, tag="X")
                nc.sync.dma_start(out=X[0:half, :], in_=x_d[p, 0:half])
                nc.gpsimd.dma_start(out=X[half:T, :], in_=x_d[p, half:T])
                absorb(X[0:1, 0:1])
                Xs.append(X)

            def stage1(p):
                """Patched row 0 via the sliding-window carry accumulation."""
                X = Xs[p]
                Wp = Wc[:, p * WW : (p + 1) * WW]
                cp = cp_pool.tile([KB, D], f32, tag="cp")
                for k in range(KB):
                    nc.tensor.matmul(
                        cp[:], lhsT=Wp[:, KB - 1 - k : 2 * KB - 1 - k],
                        rhs=X[:, k * D : (k + 1) * D],
                        start=(k == 0), stop=(k == KB - 1),
                    )
                X0n = x0npool.tile([KB, D], bf16, tag="X0n")
                if p % 2 == 0:
                    nc.vector.tensor_copy(out=X0n[:], in_=cp[:])
                else:
                    nc.scalar.copy(out=X0n[:], in_=cp[:])
                nc.gpsimd.dma_start(out=X[0:1, :], in_=X0n[:])
                absorb(X[0:1, 0:1])

            def stage2(p):
                """Within-block scan over the patched X, copy-out, store."""
                X = Xs[p]
                Ys = ypool.tile([T, KB * D], bf16, tag="Ys")
                Ap = Ac[:, p * T : (p + 1) * T]
                for g in range(G):
                    grp = gp_pool.tile([T, NG * D], f32, tag="grp")
                    sl = slice(g * NG * D, (g + 1) * NG * D)
                    nc.tensor.matmul(
                        grp[:], lhsT=Ap, rhs=X[:, sl], start=True, stop=True,
                    )
                    if g % 2 == 0:
                        nc.vector.tensor_copy(out=Ys[:, sl], in_=grp[:])
                    else:
                        nc.scalar.copy(out=Ys[:, sl], in_=grp[:])
                if p % 2 == 0:
                    nc.scalar.dma_start(out=y_d[p], in_=Ys[:])
                else:
                    nc.sync.dma_start(out=y_d[p], in_=Ys[:])

            for p in range(PAIRS + DEPTH):
                if p < PAIRS:
                    stage1(p)
                if p >= DEPTH:
                    stage2(p - DEPTH)

    # Split excess per-instruction sync waits onto InstEventSemaphore carriers.
    import bass_rust

    bass_rust.generate_event_semaphores(nc)

    _nc_cache["nc"] = nc
    return nc


def _host_constants(g):
    """Per-pair constants from float64 gamma powers."""
    pw = np.power(g, np.arange(2 * S, dtype=np.float64))
    t_idx = np.arange(T)
    t_minus_s = t_idx[None, :] - t_idx[:, None]
    A = np.where(t_minus_s >= 0, pw[np.clip(t_minus_s, 0, None)], 0.0)
    # sliding-window carry constant: for the block-k matmul, columns
    # [KB-1-k, 2KB-1-k) give W_k[t,k'] = gamma^(127-t) * gamma^(128(k'-k-1)+1)
    # for k'>k (m = k'-k+KB-1 >= KB), plus the x0 spike at (t=0, m=KB-1).
    m_idx = np.arange(WW)
    Wbig = np.zeros((T, WW))
    vals = pw[127 - t_idx][:, None] * np.where(
        m_idx[None, :] >= KB, pw[np.clip(T * (m_idx[None, :] - KB) + 1, 0, None)], 0.0
    )
    Wbig[:, :] = vals
    Wbig[0, KB - 1] = 1.0
    return A, Wbig


def _make_in_maps(tensor, gamma):
    import ml_dtypes

    bf16 = ml_dtypes.bfloat16
    x = np.ascontiguousarray(np.asarray(tensor, dtype=np.float32)).reshape(
        B * H, S, D
    )
    gam = np.asarray(gamma, dtype=np.float64).reshape(H)

    in_maps = []
    for c in range(NCORES):
        xa = np.empty((PAIRS, T, KB * D), bf16)
        A_all = np.empty((T, PAIRS * T), bf16)
        W_all = np.empty((T, PAIRS * WW), bf16)
        for p in range(PAIRS):
            pid = c * PAIRS + p
            g = gam[pid % H]
            A, Wbig = _host_constants(g)
            # x in scan layout [s, (k, d)]
            xa[p] = (
                x[pid]
                .reshape(KB, T, D)
                .transpose(1, 0, 2)
                .reshape(T, KB * D)
                .astype(bf16)
            )
            A_all[:, p * T : (p + 1) * T] = A.astype(bf16)
            W_all[:, p * WW : (p + 1) * WW] = Wbig.astype(bf16)
        in_maps.append({"x_all": xa, "A_all": A_all, "W_all": W_all})
    return in_maps


def kernel(tensor, gamma):
    from concourse.bass_utils import run_bass_kernel_spmd

    in_maps = _make_in_maps(tensor, gamma)
    nc = _build_program()
    res = run_bass_kernel_spmd(nc, in_maps, list(range(NCORES))).results
    y = np.empty((B * H, S, D), np.float32)
    for c in range(NCORES):
        yc = np.asarray(res[c]["y"]).astype(np.float32)
        y[c * PAIRS : (c + 1) * PAIRS] = (
            yc.reshape(PAIRS, T, KB, D).transpose(0, 2, 1, 3).reshape(PAIRS, S, D)
        )
    return y.reshape(B, H, S, D)


# revision 15
# speedup vs baseline: 1.6962x; 1.0882x over previous
"""Discounted cumsum (y[b,h,t,d] = x[b,h,t,d] + gamma[h] * y[b,h,t-1,d]) on 8 trn2 cores.

Blocked parallel scan, pure data parallelism over the B*H=64 (b,h) pairs (8 per core).
SBUF layout per pair: [128 part = t-within-block, 32 blocks x 128 d]. The within-block
scan is one bf16 PE matmul per 4-block group (512 moving columns) against the
triangular gamma-power matrix A[s,t] = gamma^(t-s); accumulation is fp32 in PSUM, so
the end-to-end error is ~2^-9 relative - far inside the 2e-2 gate, which lets the
whole data path run at 2 bytes/element: input x is cast to bf16 on host, output y is
written back as bf16 in the scan layout [t-within-block, block, d] (fully contiguous
1 MiB DMAs both ways); the host un-permutes and upcasts.

Carries come straight out of X with no intermediate block-sum tensor: the patched
row 0 (x'[0,k] = x[0,k] + gamma*state_{k-1}) is ONE accumulation group of 32
128-column matmuls, cp[k',d] = sum_k sum_t W_k[t,k'] x[t,(k,d)], where
W_k[t,k'] = gamma^(127-t) * gamma^(128(k'-k-1)+1) for k'>k is Toeplitz in (k'-k):
every W_k is a sliding 32-column window of one tiny per-pair constant
Wbig[t, m] = gamma^(127-t+128(m-32)+1), and the x0 term rides along as a spike
Wbig[0,31] = 1. One bf16 copy of cp and a gather back into X row 0 later, the scan
matmul over the patched X produces final y directly.

Input loads are split in half across the sync and gpsimd DMA queues, stores
alternate between the scalar and sync queues, so input and output each stream on
two queues and overlap. All loads issue up front (xin bufs=8); the PE stream is
software-pipelined (scans of pair p issue after the carry chain of pair p+2) so PE
never waits on the gather round-trip.

Walrus allows 1 sync wait on engine instructions / 2 on DMAs; after Tile scheduling,
bass_rust.generate_event_semaphores legalizes by moving excess waits onto
InstEventSemaphore carriers. The tiny bf16 ldweights "absorbers" advance PE's
observed DMA-lane clocks early so hot-path matmuls need at most their one wait.
"""

import numpy as np

B, H, S, D = 4, 16, 4096, 128
T = 128          # block length (matmul contraction dim)
KB = S // T      # 32 blocks per sequence
NG = 4           # blocks per scan-matmul group (4*128 = 512 moving columns)
G = KB // NG     # 8 scan groups per pair
WW = 2 * KB - 1  # sliding-window width of the carry constant
DEPTH = 1        # software-pipeline depth (scan of pair p after chain of p+DEPTH)
NCORES = 8
PAIRS = (B * H) // NCORES  # 8 pair-slots per core

_nc_cache = {}


def _build_program():
    if "nc" in _nc_cache:
        return _nc_cache["nc"]

    import concourse.bass as bass
    import concourse.mybir as mybir
    from concourse.tile import TileContext

    f32 = mybir.dt.float32
    bf16 = mybir.dt.bfloat16

    nc = bass.Bass(trn_type="TRN2")

    x_d = nc.declare_dram_parameter("x_all", [PAIRS, T, KB * D], bf16, isOutput=False)
    A_d = nc.declare_dram_parameter("A_all", [T, PAIRS * T], bf16, isOutput=False)
    W_d = nc.declare_dram_parameter("W_all", [T, PAIRS * WW], bf16, isOutput=False)
    y_d = nc.declare_dram_parameter("y", [PAIRS, T, KB * D], bf16, isOutput=True)

    with TileContext(nc) as tc:
        with (
            tc.tile_pool(name="const", bufs=1) as cpool,
            tc.tile_pool(name="xin", bufs=PAIRS) as xpool,
            tc.tile_pool(name="yout", bufs=3) as ypool,
            tc.tile_pool(name="x0n", bufs=2) as x0npool,
            tc.tile_pool(name="grp_ps", bufs=6, space="PSUM") as gp_pool,
            tc.tile_pool(name="c_ps", bufs=2, space="PSUM") as cp_pool,
        ):
            Ac = cpool.tile([T, PAIRS * T], bf16, tag="Ac")
            Wc = cpool.tile([T, PAIRS * WW], bf16, tag="Wc")
            nc.sync.dma_start(out=Wc[:], in_=W_d[:])
            nc.sync.dma_start(out=Ac[:], in_=A_d[:])

            def absorb(ap_src):
                # standalone bf16 ldweights: makes PE wait on that tile's DMA
                # lane here; the real matmuls self-load their own stationary.
                nc.tensor.ldweights(ap_src.bitcast(bf16))

            absorb(Ac[0:1, 0:1])
            absorb(Wc[0:1, 0:1])

            # ---- all input loads up front, halves on two queues
            Xs = []
            half = T // 2
            for p in range(PAIRS):
                X = xpool.tile([T, KB * D], bf16, tag="X")
                nc.sync.dma_start(out=X[0:half, :], in_=x_d[p, 0:half])
                nc.scalar.dma_start(out=X[half:T, :], in_=x_d[p, half:T])
                absorb(X[0:1, 0:1])
                Xs.append(X)

            def stage1(p):
                """Patched row 0 via the sliding-window carry accumulation."""
                X = Xs[p]
                Wp = Wc[:, p * WW : (p + 1) * WW]
                cp = cp_pool.tile([KB, D], f32, tag="cp")
                for k in range(KB):
                    nc.tensor.matmul(
                        cp[:], lhsT=Wp[:, KB - 1 - k : 2 * KB - 1 - k],
                        rhs=X[:, k * D : (k + 1) * D],
                        start=(k == 0), stop=(k == KB - 1),
                    )
                X0n = x0npool.tile([KB, D], bf16, tag="X0n")
                if p % 2 == 0:
                    nc.vector.tensor_copy(out=X0n[:], in_=cp[:])
                else:
                    nc.scalar.copy(out=X0n[:], in_=cp[:])
                nc.gpsimd.dma_start(out=X[0:1, :], in_=X0n[:])
                absorb(X[0:1, 0:1])

            def stage2(p):
                """Within-block scan over the patched X, copy-out, store."""
                X = Xs[p]
                Ys = ypool.tile([T, KB * D], bf16, tag="Ys")
                Ap = Ac[:, p * T : (p + 1) * T]
                for g in range(G):
                    grp = gp_pool.tile([T, NG * D], f32, tag="grp")
                    sl = slice(g * NG * D, (g + 1) * NG * D)
                    nc.tensor.matmul(
                        grp[:], lhsT=Ap, rhs=X[:, sl], start=True, stop=True,
                    )
                    if g % 2 == 0:
                        nc.vector.tensor_copy(out=Ys[:, sl], in_=grp[:])
                    else:
                        nc.scalar.copy(out=Ys[:, sl], in_=grp[:])
                    if g == G // 2 - 1 or g == G - 1:
                        # ship each column-half as soon as its copies land
                        hsl = slice((g - 3) * NG * D, (g + 1) * NG * D)
                        eng = nc.scalar if p % 2 == 0 else nc.sync
                        eng.dma_start(out=y_d[p][:, hsl], in_=Ys[:, hsl])

            for p in range(PAIRS + DEPTH):
                if p < PAIRS:
                    stage1(p)
                if p >= DEPTH:
                    stage2(p - DEPTH)

    # Split excess per-instruction sync waits onto InstEventSemaphore carriers.
    import bass_rust

    bass_rust.generate_event_semaphores(nc)

    _nc_cache["nc"] = nc
    return nc


def _host_constants(g):
    """Per-pair constants from float64 gamma powers."""
    pw = np.power(g, np.arange(2 * S, dtype=np.float64))
    t_idx = np.arange(T)
    t_minus_s = t_idx[None, :] - t_idx[:, None]
    A = np.where(t_minus_s >= 0, pw[np.clip(t_minus_s, 0, None)], 0.0)
    # sliding-window carry constant: for the block-k matmul, columns
    # [KB-1-k, 2KB-1-k) give W_k[t,k'] = gamma^(127-t) * gamma^(128(k'-k-1)+1)
    # for k'>k (m = k'-k+KB-1 >= KB), plus the x0 spike at (t=0, m=KB-1).
    m_idx = np.arange(WW)
    Wbig = np.zeros((T, WW))
    vals = pw[127 - t_idx][:, None] * np.where(
        m_idx[None, :] >= KB, pw[np.clip(T * (m_idx[None, :] - KB) + 1, 0, None)], 0.0
    )
    Wbig[:, :] = vals
    Wbig[0, KB - 1] = 1.0
    return A, Wbig


def _make_in_maps(tensor, gamma):
    import ml_dtypes

    bf16 = ml_dtypes.bfloat16
    x = np.ascontiguousarray(np.asarray(tensor, dtype=np.float32)).reshape(
        B * H, S, D
    )
    gam = np.asarray(gamma, dtype=np.float64).reshape(H)

    in_maps = []
    for c in range(NCORES):
        xa = np.empty((PAIRS, T, KB * D), bf16)
        A_all = np.empty((T, PAIRS * T), bf16)
        W_all = np.empty((T, PAIRS * WW), bf16)
        for p in range(PAIRS):
            pid = c * PAIRS + p
            g = gam[pid % H]
            A, Wbig = _host_constants(g)
            # x in scan layout [s, (k, d)]
            xa[p] = (
                x[pid]
                .reshape(KB, T, D)
                .transpose(1, 0, 2)
                .reshape(T, KB * D)
                .astype(bf16)
            )
            A_all[:, p * T : (p + 1) * T] = A.astype(bf16)
            W_all[:, p * WW : (p + 1) * WW] = Wbig.astype(bf16)
        in_maps.append({"x_all": xa, "A_all": A_all, "W_all": W_all})
    return in_maps


def kernel(tensor, gamma):
    from concourse.bass_utils import run_bass_kernel_spmd

    in_maps = _make_in_maps(tensor, gamma)
    nc = _build_program()
    res = run_bass_kernel_spmd(nc, in_maps, list(range(NCORES))).results
    y = np.empty((B * H, S, D), np.float32)
    for c in range(NCORES):
        yc = np.asarray(res[c]["y"]).astype(np.float32)
        y[c * PAIRS : (c + 1) * PAIRS] = (
            yc.reshape(PAIRS, T, KB, D).transpose(0, 2, 1, 3).reshape(PAIRS, S, D)
        )
    return y.reshape(B, H, S, D)
